# revision 16
# baseline (speedup 1.0000x reference)
"""DCT heat-blur kernel for Trainium2 (8 NeuronCores, Bass/Tile).

Math: reference computes, per image X (one (batch, channel) slice):
    coefs = D X D^T;  coefs *= E;  out = D coefs D^T
with E[h,w] = exp(-(f_h^2 + f_w^2) t_b) = e e^T rank-1.  The decay
factors through the transforms: out = M X M^T, M = D diag(e) D; the
device computes W^T X W with W = M^T built per batch on host.
2 GEMMs per image instead of 4 + an elementwise pass.

Precision classes (validated against the 2e-2 gate with >2.5x margin):
 - sigma <  6: fp16 operands; each GEMM = 4 matmuls ([K=128]x2 row
   chunks x2 output col blocks, N=256).
 - sigma >= 6: fp8 (e4m3) operands with perf_mode=DoubleRow.  The
   [partition, rowblock] layout IS DoubleRow's [Ki, Ko=2, *] interleave,
   so each GEMM collapses to 2 matmuls contracting all 256 rows at
   once -- half the PE instruction pairs.  Blur attenuates the fp8
   input noise, which is why large sigma tolerates it.
I/O is fp16 in / fp16 out for fp16-class images, fp8 in / fp16 out for
fp8-class; host casts the result back to fp32.  Per-core DMA ~13MB.

Batches are reassigned to cores so every core gets the same number of
each class (fp8 count is padded DOWN to a multiple of 8 by demoting
batches to fp16 -- always accuracy-safe), keeping the program SPMD.

Startup mitigation (the NEFF spends ~7us in engine bootstrap before any
dynamic DMA, and the PE clock-gate needs ~3.4us of sustained activity
to reach 2.4 GHz):
 1. ALL loads go on one ring in need-order, with the first W pair and
    first images as small pieces, so the first GEMM's operands complete
    ~2.5us after DMA spin-up instead of fair-sharing bandwidth.
 2. A bridge of small dummy matmuls keeps the PE busy from bootstrap
    until real data arrives, so the HAM clock-gate is already 8/8 when
    the first real GEMM issues and never re-throttles.

Sharding: pure data parallel over batch, 16 batches (48 images) per core.
"""

import os
import numpy as np

BATCH = 128
CHANNELS = 3
N = 256
N_CORES = 8
PB = BATCH // N_CORES          # batches per core
IMGS = PB * CHANNELS           # images per core
NWARM = 40                     # warmup bridge matmuls (N=128 each)
FP8_SIGMA = 6.0                # fp8 class threshold

LAST_EXEC_TIME_NS = None
_NC_CACHE = {}


def _install_ntff_hook():
    """Wire antenv.axon_hooks (missing in this image) so trace=True works."""
    import sys
    import types

    if "antenv.axon_hooks" in sys.modules:
        return
    try:
        import trn_agent_boot.trn_boot as tb

        hook = tb._ntff_profile_via_ctypes("/opt/axon/libaxon_pjrt.so")
    except Exception:
        hook = None
    m = types.ModuleType("antenv.axon_hooks")
    m.get_axon_ntff_profile_hook = lambda: hook
    m.set_axon_ntff_profile_hook = lambda h: None
    sys.modules["antenv.axon_hooks"] = m


def _img_plan(nimg, lead_halves):
    """Split nimg images into load pieces of 2 or 4, smaller pieces first."""
    plan = []
    i = 0
    for _ in range(lead_halves):
        if i + 2 <= nimg:
            plan.append((i, 2))
            i += 2
    while i < nimg:
        n = 4 if nimg - i >= 4 else nimg - i
        plan.append((i, n))
        i += n
    return plan


def _build_nc(sig):
    """sig = (nf16, nf8) batches per core per class."""
    import concourse.bacc as bacc
    import concourse.tile as tile
    import concourse.mybir as mybir

    f32 = mybir.dt.float32
    f16 = mybir.dt.float16
    f8 = mybir.dt.float8e4
    DR = mybir.MatmulPerfMode.DoubleRow

    nf16, nf8 = sig
    n16i, n8i = nf16 * CHANNELS, nf8 * CHANNELS
    assert n16i + n8i == IMGS

    nc = bacc.Bacc("TRN2", target_bir_lowering=False, debug=False)
    # per-image layout [img][partition][rowblk, col]
    tensors = {}
    if nf16:
        tensors["x16"] = nc.dram_tensor(
            "x16", [n16i, 128, 2 * N], f16, kind="ExternalInput"
        ).ap()
        tensors["w16"] = nc.dram_tensor(
            "w16", [128, nf16, 2, N], f16, kind="ExternalInput"
        ).ap()
    if nf8:
        tensors["x8"] = nc.dram_tensor(
            "x8", [n8i, 128, 2 * N], f8, kind="ExternalInput"
        ).ap()
        tensors["w8"] = nc.dram_tensor(
            "w8", [128, nf8, 2, N], f8, kind="ExternalInput"
        ).ap()
    o_d = nc.dram_tensor("o", [IMGS, 128, 2 * N], f16, kind="ExternalOutput").ap()

    plan16 = _img_plan(n16i, 4) if nf16 else []
    plan8 = _img_plan(n8i, 0) if nf8 else []

    with tile.TileContext(nc) as tc:
        with (
            tc.tile_pool(name="const", bufs=1) as cpool,
            tc.tile_pool(name="xpool", bufs=len(plan16) + len(plan8) + 1) as xpool,
            tc.tile_pool(name="tpool", bufs=6) as tpool,
            tc.tile_pool(name="opool", bufs=8) as opool,
            tc.tile_pool(name="ps1", bufs=3, space="PSUM") as ps1,
            tc.tile_pool(name="psw", bufs=1, space="PSUM") as psw,
            tc.tile_pool(name="ps2", bufs=4, space="PSUM") as ps2,
        ):
            # PE warmup bridge (see module docstring)
            wu_sb = cpool.tile([128, 128], f16, name="wu")
            nc.gpsimd.memset(wu_sb[:], 0.25)
            wu_ps = psw.tile([128, 128], f32, name="wups")
            for _ in range(NWARM):
                nc.tensor.matmul(
                    wu_ps[:], lhsT=wu_sb[:], rhs=wu_sb[:], start=True, stop=True
                )
            nc.vector.tensor_copy(out=wu_sb[:, 0:8], in_=wu_ps[:, 0:8])

            x_tiles = {}
            w_tiles = {}

            def issue_load(cls, pi):
                plan = plan16 if cls == 16 else plan8
                i0, cnt = plan[pi]
                dt_ = f16 if cls == 16 else f8
                xd = tensors["x16" if cls == 16 else "x8"]
                xt = xpool.tile(
                    [128, cnt, 2, N], dt_, tag=f"x{cls}", name=f"x{cls}_{i0}",
                    padded_shape=[128, 4, 2, N],
                )
                nc.sync.dma_start(
                    xt[:], xd[i0 : i0 + cnt].rearrange("i p (a w) -> p i a w", a=2)
                )
                x_tiles[(cls, pi)] = xt

            def issue_w(cls, q):
                nb = nf16 if cls == 16 else nf8
                cnt = min(2, nb - 2 * q)
                dt_ = f16 if cls == 16 else f8
                wd = tensors["w16" if cls == 16 else "w8"]
                wq = cpool.tile(
                    [128, cnt, 2, N], dt_, tag=f"w{cls}", name=f"w{cls}_{q}",
                    padded_shape=[128, 2, 2, N],
                )
                nc.sync.dma_start(wq[:], wd[:, 2 * q : 2 * q + cnt])
                w_tiles[(cls, q)] = wq

            # one ring, need-order: interleave W pairs between x pieces,
            # fp16 class first (it is processed first)
            pieces = []
            for cls, plan, nb in ((16, plan16, nf16), (8, plan8, nf8)):
                nw = (nb + 1) // 2
                wi = 0
                for pi, (i0, cnt) in enumerate(plan):
                    # W pair q covers images [6q, 6q+6)
                    while wi < nw and 6 * wi < i0 + cnt:
                        pieces.append(("w", cls, wi))
                        wi += 1
                    pieces.append(("x", cls, pi))
                while wi < nw:
                    pieces.append(("w", cls, wi))
                    wi += 1
            for kind, cls, idx in pieces:
                if kind == "w":
                    issue_w(cls, idx)
                else:
                    issue_load(cls, idx)

            # processing order: class-16 images then class-8 images
            imap = []
            for cls, plan in ((16, plan16), (8, plan8)):
                for pi, (i0, cnt) in enumerate(plan):
                    for ii in range(cnt):
                        imap.append((cls, pi, ii))

            ot_prev = None
            for img, (cls, pi, ii) in enumerate(imap):
                xt = x_tiles[(cls, pi)]
                b = img // CHANNELS if cls == 16 else (img - n16i) // CHANNELS
                wv = w_tiles[(cls, b // 2)][:, b % 2]
                t1_ps = ps1.tile([128, 2, N], f32, tag="p", name="t1ps")
                if cls == 16:
                    for mb in range(2):
                        for a in range(2):
                            nc.tensor.matmul(
                                t1_ps[:, mb, :],
                                lhsT=xt[:, ii, a, mb * 128 : (mb + 1) * 128],
                                rhs=wv[:, a],
                                start=(a == 0),
                                stop=(a == 1),
                            )
                else:
                    for mb in range(2):
                        nc.tensor.matmul(
                            t1_ps[:, mb, :],
                            lhsT=xt[:, ii, :, mb * 128 : (mb + 1) * 128],
                            rhs=wv[:],
                            perf_mode=DR,
                            start=True,
                            stop=True,
                        )
                t1_sb = tpool.tile(
                    [128, 2, N], f16 if cls == 16 else f8,
                    tag=f"t{cls}", name="t1sb", padded_shape=[128, 2, N],
                )
                if img % 2 == 0:
                    nc.vector.tensor_copy(out=t1_sb[:], in_=t1_ps[:])
                else:
                    nc.scalar.copy(t1_sb[:], t1_ps[:])
                t2_ps = ps2.tile([128, 2, N], f32, tag="p", name="t2ps")
                if cls == 16:
                    for mb in range(2):
                        for a in range(2):
                            nc.tensor.matmul(
                                t2_ps[:, mb, :],
                                lhsT=t1_sb[:, a, mb * 128 : (mb + 1) * 128],
                                rhs=wv[:, a],
                                start=(a == 0),
                                stop=(a == 1),
                            )
                else:
                    for mb in range(2):
                        nc.tensor.matmul(
                            t2_ps[:, mb, :],
                            lhsT=t1_sb[:, :, mb * 128 : (mb + 1) * 128],
                            rhs=wv[:],
                            perf_mode=DR,
                            start=True,
                            stop=True,
                        )
                ot = opool.tile([128, 2, N], f16, tag="o", name="ot")
                if img % 2 == 0:
                    nc.scalar.copy(ot[:], t2_ps[:])
                else:
                    nc.vector.tensor_copy(out=ot[:], in_=t2_ps[:])
                # stores: pairs -- early half on scalar, late half on
                # sync (idle after loads); the last images individually
                od = o_d[img].rearrange("p (a w) -> p a w", a=2)
                if img >= IMGS - 4:
                    nc.sync.dma_start(od, ot[:])
                elif img % 2 == 1:
                    odp = o_d[img - 1 : img + 1].rearrange(
                        "i p (a w) -> p i a w", a=2
                    )
                    ring = nc.scalar if img < IMGS // 2 else nc.sync
                    ring.dma_start(odp[:, 0], ot_prev[:])
                    ring.dma_start(odp[:, 1], ot[:])
                ot_prev = ot

    nc.compile()
    return nc


def _get_nc(sig):
    if sig not in _NC_CACHE:
        _NC_CACHE[sig] = _build_nc(sig)
    return _NC_CACHE[sig]


def _w_mats(blur_sigmas, fwd_steps):
    """Per-batch W_b = (D diag(e_b) D)^T as float64 [B, N, N] (dedup'd)."""
    sig = np.asarray(blur_sigmas, dtype=np.float64)
    steps = np.asarray(fwd_steps).astype(np.int64)
    n = np.arange(N, dtype=np.float64)
    D = np.sqrt(2.0 / N) * np.cos(np.pi * (n[None, :] + 0.5) * n[:, None] / N)
    D[0] *= 1.0 / np.sqrt(2.0)
    freqs = np.pi * n / N
    uniq, inv = np.unique(steps, return_inverse=True)
    ms = np.empty((len(uniq), N, N), dtype=np.float64)
    for i, s in enumerate(uniq):
        t = sig[s] ** 2 / 2.0
        e = np.exp(-(freqs**2) * t)
        ms[i] = (D @ (e[:, None] * D)).T
    return ms[inv]


def kernel(x, blur_sigmas, fwd_steps):
    global LAST_EXEC_TIME_NS
    import ml_dtypes
    from concourse import bass_utils

    f8np = ml_dtypes.float8_e4m3fn

    x = np.asarray(x)
    assert x.shape == (BATCH, CHANNELS, N, N), x.shape
    sigmas = np.asarray(blur_sigmas, dtype=np.float64)
    steps = np.asarray(fwd_steps).astype(np.int64)
    bsig = sigmas[steps]

    is8 = np.isfinite(bsig) & (bsig >= FP8_SIGMA)
    f8_b = [b for b in range(BATCH) if is8[b]]
    f16_b = [b for b in range(BATCH) if not is8[b]]
    # demote fp8 batches (accuracy-safe) until divisible across cores
    while len(f8_b) % N_CORES:
        f16_b.append(f8_b.pop(0))
    f16_b.sort()
    nf16, nf8 = len(f16_b) // N_CORES, len(f8_b) // N_CORES
    sig_key = (nf16, nf8)

    core_batches = []
    for c in range(N_CORES):
        core_batches.append(
            f16_b[c * nf16 : (c + 1) * nf16] + f8_b[c * nf8 : (c + 1) * nf8]
        )

    w_all = _w_mats(blur_sigmas, fwd_steps)

    def dev_img(arr, np_dt):
        # [B', C, N, N] -> [B'*C, 128, 2*N] with [img, a*128+p, w]
        nb = arr.shape[0]
        return np.ascontiguousarray(
            arr.reshape(nb * CHANNELS, 2, 128, N)
            .transpose(0, 2, 1, 3)
            .reshape(nb * CHANNELS, 128, 2 * N)
            .astype(np_dt)
        )

    def dev_w(mats, np_dt):
        # [B', N, N] -> [128, B', 2, N]
        nb = mats.shape[0]
        return np.ascontiguousarray(
            mats.astype(np_dt).reshape(nb, 2, 128, N).transpose(2, 0, 1, 3)
        )

    in_maps = []
    for c in range(N_CORES):
        bl16 = core_batches[c][:nf16]
        bl8 = core_batches[c][nf16:]
        m = {}
        if nf16:
            m["x16"] = dev_img(x[bl16], np.float16)
            m["w16"] = dev_w(w_all[bl16], np.float16)
        if nf8:
            m["x8"] = dev_img(x[bl8], f8np)
            m["w8"] = dev_w(w_all[bl8], f8np)
        in_maps.append(m)

    nc = _get_nc(sig_key)
    trace = os.environ.get("BASS_DCT_TRACE", "0") == "1"
    kwargs = {}
    if trace:
        _install_ntff_hook()
        kwargs["trace"] = True
        tmpdir = os.environ.get("BASS_DCT_TRACE_DIR")
        if tmpdir:
            kwargs["tmpdir"] = tmpdir
    res = None
    for attempt in range(3):
        try:
            res = bass_utils.run_bass_kernel_spmd(
                nc, in_maps, core_ids=list(range(N_CORES)), **kwargs
            )
            break
        except Exception:
            # transient NRT_EXEC_UNIT_UNRECOVERABLE has been observed on the
            # first execution of a freshly loaded NEFF; a retry succeeds
            if attempt == 2:
                raise
            import time as _time

            _time.sleep(2.0)
            kwargs.pop("trace", None)
            kwargs.pop("tmpdir", None)
    LAST_EXEC_TIME_NS = res.exec_time_ns

    out = np.empty((BATCH, CHANNELS, N, N), dtype=np.float32)
    for c in range(N_CORES):
        oc = res.results[c]["o"]               # [IMGS, 128, 2*N] fp16
        oi = (
            oc.reshape(PB, CHANNELS, 128, 2, N)
            .transpose(0, 1, 3, 2, 4)
            .reshape(PB, CHANNELS, N, N)
        )
        out[core_batches[c]] = oi.astype(np.float32)
    return np.ascontiguousarray(out)


# revision 17
# speedup vs baseline: 1.6809x; 1.6809x over previous
"""DCT heat-blur kernel for Trainium2 (8 NeuronCores, Bass/Tile).

Math: reference computes, per image X (one (batch, channel) slice):
    coefs = D X D^T;  coefs *= E;  out = D coefs D^T
with E[h,w] = exp(-(f_h^2 + f_w^2) t_b) = e e^T rank-1.  The elementwise
decay factors through the transforms:
    out = M X M^T,  M = D diag(e) D;  device computes W^T X W, W = M^T.
W_b is a tiny per-batch 256x256 matrix built on host.  The device does
2 GEMMs per image instead of 4 + an elementwise pass.

Device layout per 256x256 image: row-blocks a=0,1 of 128 rows each.
out = apply(apply(X, W), W) with apply(A, R) = A^T R via matmul.

Matmuls run in fp16 (full PE rate); I/O is fp16 BOTH directions -- the
host casts the fp16 result back to fp32.  Per-core DMA 21MB -> 14.7MB,
taking DMA off the critical path (PE throughput is the floor).

Startup mitigation (the NEFF spends ~7us in engine bootstrap before any
dynamic DMA, and the PE clock-gate needs ~3.4us of sustained activity to
reach 2.4 GHz):
 1. ALL loads go on one ring in need-order (W quarter 0, x groups 0-1,
    then later W quarters interleaved between x groups) so the first
    image + its W complete ~2.5us after DMA spin-up instead of fair-
    sharing bandwidth with 2MB of W needed much later.
 2. A bridge of small dummy matmuls keeps the PE busy from bootstrap
    until real data arrives, so the HAM clock-gate is already 8/8 when
    the first real GEMM issues and never re-throttles.

Sharding: pure data parallel over batch, 16 batches (48 images) per core.
"""

import os
import numpy as np

BATCH = 128
CHANNELS = 3
N = 256
N_CORES = 8
PB = BATCH // N_CORES          # batches per core
IMGS = PB * CHANNELS           # images per core
GRP = 4                        # images per DMA group
NG = IMGS // GRP               # groups per core
NWARM = 40                     # warmup bridge matmuls (N=128 each)

LAST_EXEC_TIME_NS = None
_NC_CACHE = {}


def _install_ntff_hook():
    """Wire antenv.axon_hooks (missing in this image) so trace=True works."""
    import sys
    import types

    if "antenv.axon_hooks" in sys.modules:
        return
    try:
        import trn_agent_boot.trn_boot as tb

        hook = tb._ntff_profile_via_ctypes("/opt/axon/libaxon_pjrt.so")
    except Exception:
        hook = None
    m = types.ModuleType("antenv.axon_hooks")
    m.get_axon_ntff_profile_hook = lambda: hook
    m.set_axon_ntff_profile_hook = lambda h: None
    sys.modules["antenv.axon_hooks"] = m


def _build_nc():
    import concourse.bacc as bacc
    import concourse.tile as tile
    import concourse.mybir as mybir

    f32 = mybir.dt.float32
    f16 = mybir.dt.float16

    nc = bacc.Bacc("TRN2", target_bir_lowering=False, debug=False)
    # x/o are host-permuted: [group][partition][img_in_grp, rowblk, col]
    x_d = nc.dram_tensor("x", [NG, 128, GRP * 2 * N], f16, kind="ExternalInput").ap()
    # w: host-built per-batch W matrices, [partition][batch, rowblk, col]
    w_d = nc.dram_tensor("w", [128, PB, 2, N], f16, kind="ExternalInput").ap()
    o_d = nc.dram_tensor("o", [NG, 128, GRP * 2 * N], f16, kind="ExternalOutput").ap()

    with tile.TileContext(nc) as tc:
        with (
            tc.tile_pool(name="const", bufs=1) as cpool,
            tc.tile_pool(name="xpool", bufs=NG + 1) as xpool,
            tc.tile_pool(name="tpool", bufs=6) as tpool,
            tc.tile_pool(name="opool", bufs=8) as opool,
            tc.tile_pool(name="ps1", bufs=3, space="PSUM") as ps1,
            tc.tile_pool(name="psw", bufs=1, space="PSUM") as psw,
            tc.tile_pool(name="ps2", bufs=4, space="PSUM") as ps2,
        ):
            # PE warmup bridge (see module docstring)
            wu_sb = cpool.tile([128, 128], f16, name="wu")
            nc.gpsimd.memset(wu_sb[:], 0.25)
            wu_ps = psw.tile([128, 128], f32, name="wups")
            for _ in range(NWARM):
                nc.tensor.matmul(
                    wu_ps[:], lhsT=wu_sb[:], rhs=wu_sb[:], start=True, stop=True
                )
            nc.vector.tensor_copy(out=wu_sb[:, 0:8], in_=wu_ps[:, 0:8])

            xt_tiles = {}
            w_q = {}

            def issue_load(g):
                xt = xpool.tile([128, GRP, 2, N], f16)
                nc.sync.dma_start(
                    xt[:], x_d[g].rearrange("p (i a w) -> p i a w", i=GRP, a=2)
                )
                xt_tiles[g] = xt

            def issue_wq(q):
                wq = cpool.tile([128, 2, 2, N], f16, name=f"wq{q}")
                nc.sync.dma_start(wq[:], w_d[:, 2 * q : 2 * (q + 1)])
                w_q[q] = wq

            xh_tiles = {}

            def issue_load_half(g, h):
                xt = xpool.tile(
                    [128, 2, 2, N], f16, tag="xh", name=f"xh{g}_{h}",
                    padded_shape=[128, GRP, 2, N],
                )
                nc.sync.dma_start(
                    xt[:],
                    x_d[g].rearrange("p (i a w) -> p i a w", i=GRP, a=2)[
                        :, 2 * h : 2 * h + 2
                    ],
                )
                xh_tiles[(g, h)] = xt

            # one ring, need-order: everything arrives roughly in the
            # order compute consumes it; the first pieces are halved so
            # the very first image + its W complete as early as possible
            issue_wq(0)
            issue_load_half(0, 0)
            issue_load_half(0, 1)
            issue_wq(1)
            issue_load_half(1, 0)
            issue_load_half(1, 1)
            issue_wq(2)
            issue_load(2)
            issue_wq(3)
            issue_load(3)
            issue_wq(4)
            issue_load(4)
            issue_wq(5)
            issue_load(5)
            issue_wq(6)
            issue_load(6)
            issue_wq(7)
            for g in range(7, NG):
                issue_load(g)

            for g in range(NG):
                ot = opool.tile([128, GRP, 2, N], f16)
                for ii in range(GRP):
                    img = g * GRP + ii
                    b = img // CHANNELS
                    wv = w_q[b // 2][:, b % 2]
                    if g < 2:
                        xt = xh_tiles[(g, ii // 2)][:, ii % 2 : ii % 2 + 1]
                    else:
                        xt = xt_tiles[g][:, ii : ii + 1]
                    t1_ps = ps1.tile([128, 2, N], f32)
                    for mb in range(2):
                        for a in range(2):
                            nc.tensor.matmul(
                                t1_ps[:, mb, :],
                                lhsT=xt[:, 0, a, mb * 128 : (mb + 1) * 128],
                                rhs=wv[:, a],
                                start=(a == 0),
                                stop=(a == 1),
                            )
                    t1_sb = tpool.tile([128, 2, N], f16)
                    if ii % 2 == 0:
                        nc.vector.tensor_copy(out=t1_sb[:], in_=t1_ps[:])
                    else:
                        nc.scalar.copy(t1_sb[:], t1_ps[:])
                    t2_ps = ps2.tile([128, 2, N], f32)
                    for mb in range(2):
                        for a in range(2):
                            nc.tensor.matmul(
                                t2_ps[:, mb, :],
                                lhsT=t1_sb[:, a, mb * 128 : (mb + 1) * 128],
                                rhs=wv[:, a],
                                start=(a == 0),
                                stop=(a == 1),
                            )
                    if ii % 2 == 0:
                        nc.scalar.copy(ot[:, ii], t2_ps[:])
                    else:
                        nc.vector.tensor_copy(out=ot[:, ii], in_=t2_ps[:])
                    # late groups: store per image-pair as soon as the
                    # pair is done; the last group per image.  All on the
                    # sync ring, idle once loads finish, so dispatches
                    # never queue behind other work.
                    if g == NG - 1:
                        st = nc.sync if ii % 2 == 0 else nc.scalar
                        st.dma_start(
                            o_d[g].rearrange(
                                "p (i a w) -> p i a w", i=GRP, a=2
                            )[:, ii],
                            ot[:, ii],
                        )
                    elif g >= NG // 2 and ii % 2 == 1:
                        nc.sync.dma_start(
                            o_d[g].rearrange(
                                "p (i a w) -> p i a w", i=GRP, a=2
                            )[:, ii - 1 : ii + 1],
                            ot[:, ii - 1 : ii + 1],
                        )
                if g < NG // 2:
                    nc.scalar.dma_start(
                        o_d[g].rearrange("p (i a w) -> p i a w", i=GRP, a=2), ot[:]
                    )

    nc.compile()
    return nc


def _get_nc():
    key = "nc_v9"
    if key not in _NC_CACHE:
        _NC_CACHE[key] = _build_nc()
    return _NC_CACHE[key]


def _host_w(blur_sigmas, fwd_steps):
    """Per-batch W_b = (D diag(e_b) D)^T in device layout [128, B, 2, N]."""
    sig = np.asarray(blur_sigmas, dtype=np.float64)
    steps = np.asarray(fwd_steps).astype(np.int64)
    n = np.arange(N, dtype=np.float64)
    D = np.sqrt(2.0 / N) * np.cos(np.pi * (n[None, :] + 0.5) * n[:, None] / N)
    D[0] *= 1.0 / np.sqrt(2.0)
    freqs = np.pi * n / N
    uniq, inv = np.unique(steps, return_inverse=True)
    ms = np.empty((len(uniq), N, N), dtype=np.float16)
    for i, s in enumerate(uniq):
        t = sig[s] ** 2 / 2.0
        e = np.exp(-(freqs**2) * t)
        w = (D @ (e[:, None] * D)).T
        ms[i] = w.astype(np.float16)
    w_all = ms[inv]  # [B, N, N]
    # device layout [128, B, 2, N]: [p, b, a, h] = W_b[a*128+p, h]
    return np.ascontiguousarray(
        w_all.reshape(BATCH, 2, 128, N).transpose(2, 0, 1, 3)
    )


def kernel(x, blur_sigmas, fwd_steps):
    global LAST_EXEC_TIME_NS
    from concourse import bass_utils

    x = np.asarray(x)
    assert x.shape == (BATCH, CHANNELS, N, N), x.shape
    x = x.astype(np.float16)
    w_host = _host_w(blur_sigmas, fwd_steps)

    # device x layout: [core][NG, 128, GRP*2*N]
    # x[img, a*128+p, w] -> xc[g, p, (i, a, w)]
    xp = (
        x.reshape(N_CORES, NG, GRP, 2, 128, N)
        .transpose(0, 1, 4, 2, 3, 5)
        .reshape(N_CORES, NG, 128, GRP * 2 * N)
    )
    in_maps = []
    for i in range(N_CORES):
        in_maps.append(
            {
                "x": np.ascontiguousarray(xp[i]),
                "w": np.ascontiguousarray(w_host[:, i * PB : (i + 1) * PB]),
            }
        )

    nc = _get_nc()
    trace = os.environ.get("BASS_DCT_TRACE", "0") == "1"
    kwargs = {}
    if trace:
        _install_ntff_hook()
        kwargs["trace"] = True
        tmpdir = os.environ.get("BASS_DCT_TRACE_DIR")
        if tmpdir:
            kwargs["tmpdir"] = tmpdir
    res = None
    for attempt in range(3):
        try:
            res = bass_utils.run_bass_kernel_spmd(
                nc, in_maps, core_ids=list(range(N_CORES)), **kwargs
            )
            break
        except Exception:
            # transient NRT_EXEC_UNIT_UNRECOVERABLE has been observed on the
            # first execution of a freshly loaded NEFF; a retry succeeds
            if attempt == 2:
                raise
            import time as _time

            _time.sleep(2.0)
            kwargs.pop("trace", None)
            kwargs.pop("tmpdir", None)
    LAST_EXEC_TIME_NS = res.exec_time_ns

    # inverse permute: oc[g, p, (i, a, w)] -> out[img, a*128+p, w]
    oc = np.stack([res.results[i]["o"] for i in range(N_CORES)])
    out = (
        oc.reshape(N_CORES, NG, 128, GRP, 2, N)
        .transpose(0, 1, 3, 4, 2, 5)
        .reshape(BATCH, CHANNELS, N, N)
    )
    return np.ascontiguousarray(out.astype(np.float32))
